# revision 22
# baseline (speedup 1.0000x reference)
"""Trainium2 Bass kernel for nn_MemoryNetwork (scatter_memory).

Reference computation (B=16384, I=2048, E=768, D=9, M=10, TAU=32):
    feat   = feature / ||feature||_2                       [B, I]
    mems_d = memory_tables[category[:9]]                   [D, M, E]  (first-9 quirk)
    t      = feat @ W_topic.T                              [B, E]
    att    = softmax(einsum('be,dme->bdm', t, mems_d)*TAU) [B, D, M]
    sep    = einsum('bdm,dme->bde', att, mems_d)           [B, D, E]
    dproj  = feat @ W_domain.T                             [B, E]
    out    = softmax(einsum('bde,be->bd', sep, dproj)*TAU) [B, 1, D]

Algebraic collapse (exact up to fp reassociation):
    A = mems_d.reshape(90, E) @ W_topic                    [90, I]
    C = mems_d.reshape(90, E) @ W_domain                   [90, I]
    G = feature @ [A; C].T                                 [B, 180]
    r_b = TAU / ||feature[b]||   (folded into the exps as per-row scale)
    s = G[:, :90] (topic logits, groups of 10), c = G[:, 90:]
    topic softmax over m of r*s;  L[b,d] = sum_m att*c;  out = softmax_d(r*L)

Device math runs the big GEMM in float32r (TF32-like, 11-bit mantissa,
1.0 PE cycles/row when the moving free dim is >= 256 -- 4x faster than
fp32).  Raw f32r rounding of both operands costs ~2.3e-2 rel err (over
the 2e-2 gate), so the topic block gets a first-order correction: the
moving tensor is [A_r | dA_r | C_r] (270 cols) where dA_r = f32r(A - A_r),
and s = P[:,:90] + P[:,90:180] is folded during PSUM eviction.  Measured
end-to-end rel err ~1.4e-2.

Per core: 256 matmuls x 270 moving cols = 69,120 PE cycles (~28.8us).
Feature tiles are pre-rounded to the f32r grid AND pre-transposed on the
host (so no PE transposes at all), then streamed as [128, 512] slices
across three DMA queues (sync/HWDGE, scalar/HWDGE, gpsimd/SWDGE).
Sharding: data-parallel over B across 8 cores; K / memory tables are
host-replicated (tiny).
"""

import os
import sys

import numpy as np

for _p in ("/opt/trn_rl_repo", "/root/.axon_site/_ro/trn_rl_repo"):
    if os.path.isdir(_p) and _p not in sys.path:
        sys.path.insert(0, _p)

B, I, E = 16384, 2048, 768
D, M, TAU = 9, 10, 32.0
NCORES = 8
BLOC = B // NCORES          # 2048 rows per core
KI = I // 128               # 16 contraction blocks
SC = D * M                  # 90 = one logit block
KC = 3 * SC                 # 270 moving cols: [A_r | dA_r | C_r]
CHUNK = 512                 # batch-chunk
NCHUNK = BLOC // CHUNK      # 4
NTPC = CHUNK // 128         # 4 b-tiles per chunk
NT = BLOC // 128            # 16 b-tiles per core
CSMALL = NT                 # [r] fp32 const

_NC_CACHE = {}


def _build_nc():
    import concourse.bass as bass
    import concourse.mybir as mybir
    import concourse.tile as tile

    fp32 = mybir.dt.float32
    f32r = mybir.dt.float32r
    Alu = mybir.AluOpType
    Act = mybir.ActivationFunctionType

    nc = bass.Bass()
    # Pre-rounded feature, transposed per core: feat[i*128+p, b] =
    # f32r(feature)[core*BLOC + b, i*128 + p].
    feat = nc.dram_tensor("feat", [KI * 128, BLOC], f32r,
                          kind="ExternalInput")
    # K moving tensor, f32r grid: [128, KI, 270].
    ktr = nc.dram_tensor("ktr", [128, KI * KC], f32r, kind="ExternalInput")
    # Small fp32 consts: r[p, t] = TAU/||feature[t*128+p]||.
    cst = nc.dram_tensor("cst", [128, CSMALL], fp32, kind="ExternalInput")
    out = nc.dram_tensor("out", [BLOC, D], fp32, kind="ExternalOutput")

    with tile.TileContext(nc) as tc:
        with (
            tc.tile_pool(name="const", bufs=1) as cpool,
            tc.tile_pool(name="stp", bufs=2) as stp,
            tc.tile_pool(name="pG", bufs=1, space="PSUM") as pG,
        ):
            ot_all = cpool.tile([128, NT, D], fp32)
            cst_sb = cpool.tile([128, CSMALL], fp32)
            ktr_sb = cpool.tile([128, KI, KC], f32r)
            ftT = [cpool.tile([128, KI, CHUNK], f32r, name=f"ftT{c}")
                   for c in range(NCHUNK)]
            sc_t = [cpool.tile([128, NTPC, 2 * SC], fp32, name=f"sc{c}")
                    for c in range(NCHUNK)]
            r_sb = cst_sb[:, :NT]
            kv = ktr[:, :].rearrange("p (k c) -> p k c", c=KC)
            wjk = cpool.tile([128, 128], fp32)

            # ---- DMA issue plan ----------------------------------------
            # Cold-start need order: ktr[i] and slice(c0, i) in ascending i
            # (PSUM accumulation makes any i order legal, arrival ~matches
            # ascending).  Queues: sync (HWDGE), scalar (HWDGE), gpsimd
            # (SWDGE, ~1us/DMA descriptor-gen -> few coarse pieces).
            def fslice(c, i0, i1, eng):
                src = feat[i0 * 128:i1 * 128, c * CHUNK:(c + 1) * CHUNK]
                src = src.rearrange("(k p) b -> p k b", p=128)
                eng.dma_start(ftT[c][:, i0:i1, :], src)

            # memset must be the FIRST pool-sequencer instruction or it
            # queues behind the SWDGE descriptor-gens and the PE warm-up
            # never happens.
            nc.gpsimd.memset(wjk[:, :], 0.0)
            nc.sync.dma_start(cst_sb, cst[:, :])
            KQ = ((0, 1), (1, 2), (2, 4), (4, 6), (6, 8), (8, 10),
                  (10, 12), (12, 14), (14, 16))
            for q0, q1 in KQ:
                nc.sync.dma_start(ktr_sb[:, q0:q1, :], kv[:, q0:q1, :])
            # scalar (= Act engine) only carries pieces needed EARLY: its
            # DMA-queue backpressure convoys the Act compute ops behind it.
            for i0, i1 in ((0, 1), (1, 2), (2, 3), (3, 4), (4, 6)):
                fslice(0, i0, i1, nc.scalar)
            for i0, i1 in ((6, 8), (8, 10), (10, 12), (12, 14)):
                fslice(0, i0, i1, nc.gpsimd)
            fslice(0, 14, 16, nc.sync)
            fslice(1, 0, 4, nc.scalar)
            fslice(1, 4, 8, nc.sync)
            fslice(1, 8, 12, nc.gpsimd)
            fslice(1, 12, 16, nc.gpsimd)
            for c in range(2, NCHUNK):
                fslice(c, 0, 4, nc.sync)
                fslice(c, 4, 8, nc.sync)
                fslice(c, 8, 12, nc.gpsimd)
                fslice(c, 12, 16, nc.scalar)

            # ---- PSUM: 8 banks = 2 chunk-parities x 4 b-tiles ----------
            gp = [[pG.tile([128, KC], fp32, tag=f"gp{par}{j}",
                           name=f"gp{par}{j}") for j in range(NTPC)]
                  for par in range(2)]

            # ---- PE warm-up --------------------------------------------
            # Warm the PE p-state ramp on the memset junk tile (no DMA in
            # the way).  Targets chunk 0's own PSUM bank: the WAW
            # dependency anchors the warm-up BEFORE the first real matmul
            # -- junk with no consumers gets parked late by the scheduler
            # and convoys its engine.  (Multi-wait instructions are handled
            # by the InstDrain post-pass, so no sem-absorber junk needed.)
            for w in range(10):
                nc.tensor.transpose(gp[0][0][:, :128], wjk, wjk)

            def evict(c, j):
                # DVE can read only ONE non-scalar PSUM input per op: stage
                # the dA_r half through SBUF on the (idle) Act engine first.
                # order matters for the walrus single-sync-wait limit: the
                # DVE c-copy and the Act tmp-copy each consume the PE stop
                # sem on their engine, so the final add waits on Act only.
                g = gp[c % 2][j]
                tmp = stp.tile([128, SC], fp32, tag="sdl", name="sdl")
                nc.vector.tensor_copy(sc_t[c][:, j, SC:2 * SC],
                                      g[:, 2 * SC:3 * SC])
                nc.scalar.activation(tmp, g[:, SC:2 * SC], Act.Copy)
                nc.vector.tensor_tensor(sc_t[c][:, j, 0:SC], g[:, 0:SC],
                                        tmp, Alu.add)

            def softmax_chunk(ci, j0=0, j1=NTPC):
                sc = sc_t[ci]
                nj = j1 - j0
                S = sc[:, j0:j1, 0:SC].rearrange("p c (d m) -> p c d m", m=M)
                Cv = sc[:, j0:j1, SC:2 * SC].rearrange(
                    "p c (d m) -> p c d m", m=M)
                sh4 = (128, nj, D, M)
                mx = stp.tile([128, nj, D], fp32, tag="mx", name="mx")
                nc.vector.tensor_reduce(mx, S, axis=mybir.AxisListType.X,
                                        op=Alu.max)
                nc.vector.tensor_tensor(
                    S, S, mx[:, :, :, None].to_broadcast(sh4), Alu.subtract)
                ex = stp.tile([128, nj, D, M], fp32, tag="ex", name="ex")
                for j in range(j0, j1):
                    t = ci * NTPC + j
                    nc.scalar.activation(
                        ex[:, j - j0], sc[:, j, 0:SC].rearrange(
                            "p (d m) -> p d m", m=M),
                        Act.Exp, scale=r_sb[:, t:t + 1])
                den = stp.tile([128, nj, D], fp32, tag="den", name="den")
                nc.vector.tensor_reduce(den, ex, axis=mybir.AxisListType.X,
                                        op=Alu.add)
                ec = stp.tile([128, nj, D, M], fp32, tag="ec", name="ec")
                nc.vector.tensor_tensor(ec, ex, Cv, Alu.mult)
                num = stp.tile([128, nj, D], fp32, tag="num", name="num")
                nc.vector.tensor_reduce(num, ec, axis=mybir.AxisListType.X,
                                        op=Alu.add)
                rden = stp.tile([128, nj, D], fp32, tag="rden", name="rden")
                nc.vector.reciprocal(rden, den)
                L = stp.tile([128, nj, D], fp32, tag="L", name="L")
                nc.vector.tensor_tensor(L, num, rden, Alu.mult)
                sh3 = (128, nj, D)
                # domain softmax: r*L stays within +-~85, exp() cannot
                # overflow fp32 -- skip the max-subtraction
                e2 = stp.tile([128, nj, D], fp32, tag="e2", name="e2")
                for j in range(j0, j1):
                    t = ci * NTPC + j
                    nc.scalar.activation(e2[:, j - j0], L[:, j - j0],
                                         Act.Exp, scale=r_sb[:, t:t + 1])
                den2 = stp.tile([128, nj], fp32, tag="den2", name="den2")
                nc.vector.tensor_reduce(den2, e2, axis=mybir.AxisListType.X,
                                        op=Alu.add)
                rden2 = stp.tile([128, nj], fp32, tag="rden2", name="rden2")
                nc.vector.reciprocal(rden2, den2)
                nc.vector.tensor_tensor(
                    ot_all[:, ci * NTPC + j0:ci * NTPC + j1, :], e2,
                    rden2[:, :, None].to_broadcast(sh3), Alu.mult)

            outv = out[:, :].rearrange("(t p) d -> p t d", p=128)

            # ---- chunks 0-2: i-outer (stream slices as they arrive) ----
            for c in range(NCHUNK - 1):
                par = c % 2
                if c >= 2:
                    # absorb the PSUM WAR-release wait (vs chunk c-2's
                    # eviction) into junk matmuls, one per bank
                    for j in range(NTPC):
                        nc.tensor.matmul(gp[par][j][:1, :1],
                                         ktr_sb[:, 0, :1].bitcast(fp32),
                                         ktr_sb[:, 0, :1].bitcast(fp32))
                for i in range(KI):
                    for j in range(NTPC):
                        nc.tensor.matmul(
                            gp[par][j],
                            ftT[c][:, i, j * 128:(j + 1) * 128],
                            ktr_sb[:, i, :],
                            start=(i == 0), stop=(i == KI - 1))
                if c > 0:
                    # previous chunk's softmax overlaps this chunk's PE work
                    softmax_chunk(c - 1)
                for j in range(NTPC):
                    evict(c, j)

            # ---- last chunk: j-outer so evictions/softmax pipeline -----
            # Tail-latency-optimized per-b-tile chain: topic max comes
            # straight from PSUM P[:, :90] (the dA_r part only shifts it by
            # ~2^-12, and the max-subtract needs only an approximate max
            # for stability), in parallel with the Act-staged fold; the
            # C block is read from PSUM directly (no staging copy).
            # Per-quarter chain, split into stages so four chains can be
            # software-pipelined across the j-groups (DVE executes its
            # stream in-order; un-interleaved chains serialize on their
            # cross-engine latency gaps).  Topic max comes straight from
            # PSUM P[:, :90] (the dA_r part only shifts it by ~2^-12 and
            # stabilization needs only an approximate max), in parallel
            # with the Act-staged fold; the C block is read from PSUM.
            qt = [dict() for _ in range(NTPC)]

            def q_s1(c, j):
                g = gp[c % 2][j]
                Sg = sc_t[c][:, j, 0:SC].rearrange("p (d m) -> p d m", m=M)
                mx = stp.tile([128, D], fp32, tag="qmx", name="qmx")
                nc.vector.tensor_reduce(
                    mx, g[:, 0:SC].rearrange("p (d m) -> p d m", m=M),
                    axis=mybir.AxisListType.X, op=Alu.max)
                tmp = stp.tile([128, SC], fp32, tag="sdl", name="sdl")
                nc.scalar.activation(tmp, g[:, SC:2 * SC], Act.Copy)
                nc.vector.tensor_tensor(sc_t[c][:, j, 0:SC], g[:, 0:SC],
                                        tmp, Alu.add)
                nc.vector.tensor_tensor(
                    Sg, Sg, mx[:, :, None].to_broadcast((128, D, M)),
                    Alu.subtract)
                qt[j]["Sg"] = Sg

            def q_s2(c, j):
                t = c * NTPC + j
                ex = stp.tile([128, D, M], fp32, tag="qex", name="qex")
                nc.scalar.activation(ex, qt[j]["Sg"], Act.Exp,
                                     scale=r_sb[:, t:t + 1])
                qt[j]["ex"] = ex

            def q_s3(c, j):
                g = gp[c % 2][j]
                ex = qt[j]["ex"]
                den = stp.tile([128, D], fp32, tag="qden", name="qden")
                nc.vector.tensor_reduce(den, ex, axis=mybir.AxisListType.X,
                                        op=Alu.add)
                ec = stp.tile([128, D, M], fp32, tag="qec", name="qec")
                nc.vector.tensor_tensor(
                    ec, ex, g[:, 2 * SC:3 * SC].rearrange(
                        "p (d m) -> p d m", m=M), Alu.mult)
                num = stp.tile([128, D], fp32, tag="qnum", name="qnum")
                nc.vector.tensor_reduce(num, ec, axis=mybir.AxisListType.X,
                                        op=Alu.add)
                rden = stp.tile([128, D], fp32, tag="qrden", name="qrden")
                nc.vector.reciprocal(rden, den)
                L = stp.tile([128, D], fp32, tag="qL", name="qL")
                nc.vector.tensor_tensor(L, num, rden, Alu.mult)
                qt[j]["L"] = L

            def q_s4(c, j):
                t = c * NTPC + j
                e2 = stp.tile([128, D], fp32, tag="qe2", name="qe2")
                nc.scalar.activation(e2, qt[j]["L"], Act.Exp,
                                     scale=r_sb[:, t:t + 1])
                qt[j]["e2"] = e2

            def q_s5(c, j):
                t = c * NTPC + j
                e2 = qt[j]["e2"]
                den2 = stp.tile([128, 1], fp32, tag="qden2", name="qden2")
                nc.vector.tensor_reduce(den2, e2, axis=mybir.AxisListType.X,
                                        op=Alu.add)
                rden2 = stp.tile([128, 1], fp32, tag="qrden2",
                                 name="qrden2")
                nc.vector.reciprocal(rden2, den2)
                nc.vector.tensor_scalar_mul(ot_all[:, t, :], e2, rden2)

            c = NCHUNK - 1
            par = c % 2
            for j in range(NTPC):
                nc.tensor.matmul(gp[par][j][:1, :1],
                                 ktr_sb[:, 0, :1].bitcast(fp32),
                                 ktr_sb[:, 0, :1].bitcast(fp32))
            for j in range(NTPC):
                for i in range(KI):
                    nc.tensor.matmul(
                        gp[par][j],
                        ftT[c][:, i, j * 128:(j + 1) * 128],
                        ktr_sb[:, i, :],
                        start=(i == 0), stop=(i == KI - 1))
                if j == 0:
                    softmax_chunk(c - 1)
                elif j == 1:
                    q_s1(c, 0)
                    q_s2(c, 0)
                elif j == 2:
                    q_s1(c, 1)
                    q_s3(c, 0)
                    q_s2(c, 1)
                    q_s4(c, 0)
                else:
                    q_s1(c, 2)
                    q_s3(c, 1)
                    q_s5(c, 0)
                    q_s2(c, 2)
                    q_s4(c, 1)
                    nc.sync.dma_start(outv[:, :NT - 3, :],
                                      ot_all[:, :NT - 3, :])
            q_s1(c, 3)
            q_s3(c, 2)
            q_s5(c, 1)
            q_s2(c, 3)
            q_s4(c, 2)
            q_s3(c, 3)
            q_s5(c, 2)
            q_s4(c, 3)
            q_s5(c, 3)
            nc.sync.dma_start(outv[:, NT - 3:, :], ot_all[:, NT - 3:, :])

    # Post-pass: walrus's codegen rejects instructions with more than one
    # embedded sync wait (S3_LW single-slot limit).  For ANY instruction
    # carrying N>1 waits, hoist N-1 of them into single-wait InstDrain
    # sequencer ops on the same engine immediately before it; the sequencer
    # consumes them in order, so semantics are identical.
    for fn in nc.m.functions:
        for blk in fn.blocks:
            lst = blk.instructions
            k = 0
            while k < len(lst):
                ins = lst[k]
                si = ins.sync_info
                if si is not None and si.on_wait and len(si.on_wait) > 1:
                    w = list(si.on_wait)
                    ups = list(si.on_update or [])
                    ins.sync_info = mybir.SyncInfo(on_wait=[w[-1]],
                                                   on_update=ups)
                    for j, wx in enumerate(w[:-1]):
                        lst.insert(k + j, mybir.InstDrain(
                            name=f"{ins.name}-sw{j}", engine=ins.engine,
                            sync_info=mybir.SyncInfo(on_wait=[wx],
                                                     on_update=[])))
                    k += len(w) - 1
                k += 1

    return nc


def _get_nc():
    if "nc" not in _NC_CACHE:
        _NC_CACHE["nc"] = _build_nc()
    return _NC_CACHE["nc"]


def _rne11(x):
    """Round fp32 to the f32r grid (11 explicit mantissa bits)."""
    xv = np.ascontiguousarray(x, dtype=np.float32).view(np.uint32)
    xv = xv.astype(np.uint64)
    out = ((xv + np.uint64(0x800)) & np.uint64(0xFFFFF000)).astype(np.uint32)
    return out.view(np.float32)


def _host_prep(feature, W_topic, W_domain, memory_tables, category):
    feature = np.ascontiguousarray(np.asarray(feature, dtype=np.float32))
    cat = np.asarray(category).astype(np.int64)
    mems = np.asarray(memory_tables, dtype=np.float32)[cat[:D]]       # [9,10,768]
    mf = mems.reshape(D * M, E).astype(np.float64)
    A = (mf @ np.asarray(W_topic, dtype=np.float64)).astype(np.float32)
    C = (mf @ np.asarray(W_domain, dtype=np.float64)).astype(np.float32)
    Ar = _rne11(A)
    dAr = _rne11(A - Ar)
    Cr = _rne11(C)
    # ktr[p, i, 0:90]=Ar.T, 90:180=dAr.T, 180:270=Cr.T (per i-block of I)
    KT = np.empty((128, KI, KC), dtype=np.float32)
    KT[:, :, 0:SC] = Ar.T.reshape(KI, 128, SC).transpose(1, 0, 2)
    KT[:, :, SC:2 * SC] = dAr.T.reshape(KI, 128, SC).transpose(1, 0, 2)
    KT[:, :, 2 * SC:3 * SC] = Cr.T.reshape(KI, 128, SC).transpose(1, 0, 2)
    KT = np.ascontiguousarray(KT).reshape(128, KI * KC)

    norm = np.sqrt(np.einsum("bi,bi->b", feature, feature,
                             dtype=np.float64))
    r = (TAU / np.maximum(norm, 1e-12)).astype(np.float32)            # [B]
    cst = np.ascontiguousarray(
        r.reshape(NCORES, BLOC // 128, 128).transpose(0, 2, 1))

    # featT[core] = f32r(feature[core block]).T  -> [I, BLOC]
    fr = _rne11(feature)
    featT = np.ascontiguousarray(
        fr.reshape(NCORES, BLOC, I).transpose(0, 2, 1))
    return featT, (cst, KT)


def _run(featT, cstpack, trace=False):
    from concourse.bass_utils import run_bass_kernel_spmd

    cst, KT = cstpack
    nc = _get_nc()
    in_maps = [
        {"feat": featT[c], "cst": cst[c], "ktr": KT}
        for c in range(NCORES)
    ]
    res = run_bass_kernel_spmd(nc, in_maps, core_ids=list(range(NCORES)),
                               trace=trace)
    out = np.concatenate([r["out"] for r in res.results], axis=0)     # [B, 9]
    return out.reshape(B, 1, D), res


def kernel(feature=None, W_topic=None, W_domain=None, memory_tables=None,
           category=None, **_unused):
    featT, cstpack = _host_prep(feature, W_topic, W_domain, memory_tables,
                                category)
    out, _ = _run(featT, cstpack, trace=False)
    return out


# revision 24
# speedup vs baseline: 1.0177x; 1.0177x over previous
"""Trainium2 Bass kernel for nn_MemoryNetwork (scatter_memory).

Reference computation (B=16384, I=2048, E=768, D=9, M=10, TAU=32):
    feat   = feature / ||feature||_2                       [B, I]
    mems_d = memory_tables[category[:9]]                   [D, M, E]  (first-9 quirk)
    t      = feat @ W_topic.T                              [B, E]
    att    = softmax(einsum('be,dme->bdm', t, mems_d)*TAU) [B, D, M]
    sep    = einsum('bdm,dme->bde', att, mems_d)           [B, D, E]
    dproj  = feat @ W_domain.T                             [B, E]
    out    = softmax(einsum('bde,be->bd', sep, dproj)*TAU) [B, 1, D]

Algebraic collapse (exact up to fp reassociation):
    A = mems_d.reshape(90, E) @ W_topic                    [90, I]
    C = mems_d.reshape(90, E) @ W_domain                   [90, I]
    G = feature @ [A; C].T                                 [B, 180]
    r_b = TAU / ||feature[b]||   (folded into the exps as per-row scale)
    s = G[:, :90] (topic logits, groups of 10), c = G[:, 90:]
    topic softmax over m of r*s;  L[b,d] = sum_m att*c;  out = softmax_d(r*L)

Device math runs the big GEMM in float32r (TF32-like, 11-bit mantissa,
1.0 PE cycles/row when the moving free dim is >= 256 -- 4x faster than
fp32).  Raw f32r rounding of both operands costs ~2.3e-2 rel err (over
the 2e-2 gate), so the topic block gets a first-order correction: the
moving tensor is [A_r | dA_r | C_r] (270 cols) where dA_r = f32r(A - A_r),
and s = P[:,:90] + P[:,90:180] is folded during PSUM eviction.  Measured
end-to-end rel err ~1.4e-2.

Per core: 256 matmuls x 270 moving cols = 69,120 PE cycles (~28.8us).
Feature tiles are pre-rounded to the f32r grid AND pre-transposed on the
host (so no PE transposes at all), then streamed as [128, 512] slices
across three DMA queues (sync/HWDGE, scalar/HWDGE, gpsimd/SWDGE).
Sharding: data-parallel over B across 8 cores; K / memory tables are
host-replicated (tiny).
"""

import os
import sys

import numpy as np

for _p in ("/opt/trn_rl_repo", "/root/.axon_site/_ro/trn_rl_repo"):
    if os.path.isdir(_p) and _p not in sys.path:
        sys.path.insert(0, _p)

B, I, E = 16384, 2048, 768
D, M, TAU = 9, 10, 32.0
NCORES = 8
BLOC = B // NCORES          # 2048 rows per core
KI = I // 128               # 16 contraction blocks
SC = D * M                  # 90 = one logit block
KC = 3 * SC                 # 270 moving cols: [A_r | dA_r | C_r]
CHUNK = 512                 # batch-chunk
NCHUNK = BLOC // CHUNK      # 4
NTPC = CHUNK // 128         # 4 b-tiles per chunk
NT = BLOC // 128            # 16 b-tiles per core
CSMALL = NT                 # [r] fp32 const

_NC_CACHE = {}


def _build_nc():
    import concourse.bass as bass
    import concourse.mybir as mybir
    import concourse.tile as tile

    fp32 = mybir.dt.float32
    f32r = mybir.dt.float32r
    Alu = mybir.AluOpType
    Act = mybir.ActivationFunctionType

    nc = bass.Bass()
    # Pre-rounded feature, transposed per core: feat[i*128+p, b] =
    # f32r(feature)[core*BLOC + b, i*128 + p].
    feat = nc.dram_tensor("feat", [KI * 128, BLOC], f32r,
                          kind="ExternalInput")
    # K moving tensor, f32r grid: [128, KI, 270].
    ktr = nc.dram_tensor("ktr", [128, KI * KC], f32r, kind="ExternalInput")
    # Small fp32 consts: r[p, t] = TAU/||feature[t*128+p]||.
    cst = nc.dram_tensor("cst", [128, CSMALL], fp32, kind="ExternalInput")
    out = nc.dram_tensor("out", [BLOC, D], fp32, kind="ExternalOutput")

    with tile.TileContext(nc) as tc:
        with (
            tc.tile_pool(name="const", bufs=1) as cpool,
            tc.tile_pool(name="stp", bufs=2) as stp,
            tc.tile_pool(name="pG", bufs=1, space="PSUM") as pG,
        ):
            ot_all = cpool.tile([128, NT, D], fp32)
            cst_sb = cpool.tile([128, CSMALL], fp32)
            ktr_sb = cpool.tile([128, KI, KC], f32r)
            ftT = [cpool.tile([128, KI, CHUNK], f32r, name=f"ftT{c}")
                   for c in range(NCHUNK)]
            sc_t = [cpool.tile([128, NTPC, 2 * SC], fp32, name=f"sc{c}")
                    for c in range(NCHUNK)]
            r_sb = cst_sb[:, :NT]
            kv = ktr[:, :].rearrange("p (k c) -> p k c", c=KC)
            wjk = cpool.tile([128, 128], fp32)

            # ---- DMA issue plan ----------------------------------------
            # Cold-start need order: ktr[i] and slice(c0, i) in ascending i
            # (PSUM accumulation makes any i order legal, arrival ~matches
            # ascending).  Queues: sync (HWDGE), scalar (HWDGE), gpsimd
            # (SWDGE, ~1us/DMA descriptor-gen -> few coarse pieces).
            def fslice(c, i0, i1, eng):
                src = feat[i0 * 128:i1 * 128, c * CHUNK:(c + 1) * CHUNK]
                src = src.rearrange("(k p) b -> p k b", p=128)
                eng.dma_start(ftT[c][:, i0:i1, :], src)

            # memset must be the FIRST pool-sequencer instruction or it
            # queues behind the SWDGE descriptor-gens and the PE warm-up
            # never happens.
            nc.gpsimd.memset(wjk[:, :], 0.0)
            nc.sync.dma_start(cst_sb, cst[:, :])
            KQ = ((0, 1), (1, 2), (2, 4), (4, 6), (6, 8), (8, 10),
                  (10, 12), (12, 14), (14, 16))
            for q0, q1 in KQ:
                nc.sync.dma_start(ktr_sb[:, q0:q1, :], kv[:, q0:q1, :])
            # scalar (= Act engine) only carries pieces needed EARLY: its
            # DMA-queue backpressure convoys the Act compute ops behind it.
            for i0, i1 in ((0, 1), (1, 2), (2, 3), (3, 4), (4, 6)):
                fslice(0, i0, i1, nc.scalar)
            for i0, i1 in ((6, 8), (8, 10), (10, 12), (12, 14)):
                fslice(0, i0, i1, nc.gpsimd)
            fslice(0, 14, 16, nc.sync)
            fslice(1, 0, 4, nc.scalar)
            fslice(1, 4, 8, nc.sync)
            fslice(1, 8, 12, nc.gpsimd)
            fslice(1, 12, 16, nc.gpsimd)
            for c in range(2, NCHUNK):
                fslice(c, 0, 4, nc.sync)
                fslice(c, 4, 8, nc.sync)
                fslice(c, 8, 12, nc.gpsimd)
                fslice(c, 12, 16, nc.scalar)

            # ---- PSUM: 8 banks = 2 chunk-parities x 4 b-tiles ----------
            gp = [[pG.tile([128, KC], fp32, tag=f"gp{par}{j}",
                           name=f"gp{par}{j}") for j in range(NTPC)]
                  for par in range(2)]

            # ---- PE warm-up --------------------------------------------
            # Warm the PE p-state ramp on the memset junk tile (no DMA in
            # the way).  Targets chunk 0's own PSUM bank: the WAW
            # dependency anchors the warm-up BEFORE the first real matmul
            # -- junk with no consumers gets parked late by the scheduler
            # and convoys its engine.  (Multi-wait instructions are handled
            # by the InstDrain post-pass, so no sem-absorber junk needed.)
            for w in range(10):
                nc.tensor.transpose(gp[0][0][:, :128], wjk, wjk)

            def evict(c, j):
                # DVE can read only ONE non-scalar PSUM input per op: stage
                # the dA_r half through SBUF on the (idle) Act engine, which
                # also evicts the C block -- the DVE does only the fold-add
                # (it is the throughput-critical engine).
                g = gp[c % 2][j]
                tmp = stp.tile([128, SC], fp32, tag="sdl", name="sdl")
                nc.scalar.activation(sc_t[c][:, j, SC:2 * SC],
                                     g[:, 2 * SC:3 * SC], Act.Copy)
                nc.scalar.activation(tmp, g[:, SC:2 * SC], Act.Copy)
                nc.vector.tensor_tensor(sc_t[c][:, j, 0:SC], g[:, 0:SC],
                                        tmp, Alu.add)

            def softmax_chunk(ci, j0=0, j1=NTPC):
                sc = sc_t[ci]
                nj = j1 - j0
                S = sc[:, j0:j1, 0:SC].rearrange("p c (d m) -> p c d m", m=M)
                Cv = sc[:, j0:j1, SC:2 * SC].rearrange(
                    "p c (d m) -> p c d m", m=M)
                sh4 = (128, nj, D, M)
                mx = stp.tile([128, nj, D], fp32, tag="mx", name="mx")
                nc.vector.tensor_reduce(mx, S, axis=mybir.AxisListType.X,
                                        op=Alu.max)
                nc.vector.tensor_tensor(
                    S, S, mx[:, :, :, None].to_broadcast(sh4), Alu.subtract)
                ex = stp.tile([128, nj, D, M], fp32, tag="ex", name="ex")
                for j in range(j0, j1):
                    t = ci * NTPC + j
                    nc.scalar.activation(
                        ex[:, j - j0], sc[:, j, 0:SC].rearrange(
                            "p (d m) -> p d m", m=M),
                        Act.Exp, scale=r_sb[:, t:t + 1])
                den = stp.tile([128, nj, D], fp32, tag="den", name="den")
                nc.vector.tensor_reduce(den, ex, axis=mybir.AxisListType.X,
                                        op=Alu.add)
                ec = stp.tile([128, nj, D, M], fp32, tag="ec", name="ec")
                # the elementwise multiply runs on the (otherwise idle)
                # gpsimd engine to keep the DVE stream short
                nc.gpsimd.tensor_tensor(ec, ex, Cv, Alu.mult)
                num = stp.tile([128, nj, D], fp32, tag="num", name="num")
                nc.vector.tensor_reduce(num, ec, axis=mybir.AxisListType.X,
                                        op=Alu.add)
                rden = stp.tile([128, nj, D], fp32, tag="rden", name="rden")
                nc.vector.reciprocal(rden, den)
                L = stp.tile([128, nj, D], fp32, tag="L", name="L")
                nc.vector.tensor_tensor(L, num, rden, Alu.mult)
                sh3 = (128, nj, D)
                # domain softmax: r*L stays within +-~85, exp() cannot
                # overflow fp32 -- skip the max-subtraction
                e2 = stp.tile([128, nj, D], fp32, tag="e2", name="e2")
                for j in range(j0, j1):
                    t = ci * NTPC + j
                    nc.scalar.activation(e2[:, j - j0], L[:, j - j0],
                                         Act.Exp, scale=r_sb[:, t:t + 1])
                den2 = stp.tile([128, nj], fp32, tag="den2", name="den2")
                nc.vector.tensor_reduce(den2, e2, axis=mybir.AxisListType.X,
                                        op=Alu.add)
                rden2 = stp.tile([128, nj], fp32, tag="rden2", name="rden2")
                nc.vector.reciprocal(rden2, den2)
                nc.vector.tensor_tensor(
                    ot_all[:, ci * NTPC + j0:ci * NTPC + j1, :], e2,
                    rden2[:, :, None].to_broadcast(sh3), Alu.mult)

            outv = out[:, :].rearrange("(t p) d -> p t d", p=128)

            # ---- chunks 0-2: i-outer (stream slices as they arrive) ----
            for c in range(NCHUNK - 1):
                par = c % 2
                if c >= 2:
                    # absorb the PSUM WAR-release wait (vs chunk c-2's
                    # eviction) into junk matmuls, one per bank
                    for j in range(NTPC):
                        nc.tensor.matmul(gp[par][j][:1, :1],
                                         ktr_sb[:, 0, :1].bitcast(fp32),
                                         ktr_sb[:, 0, :1].bitcast(fp32))
                for i in range(KI):
                    for j in range(NTPC):
                        nc.tensor.matmul(
                            gp[par][j],
                            ftT[c][:, i, j * 128:(j + 1) * 128],
                            ktr_sb[:, i, :],
                            start=(i == 0), stop=(i == KI - 1))
                if c > 0:
                    # previous chunk's softmax overlaps this chunk's PE work
                    softmax_chunk(c - 1)
                for j in range(NTPC):
                    evict(c, j)

            # ---- last chunk: j-outer so evictions/softmax pipeline -----
            # Tail-latency-optimized per-b-tile chain: topic max comes
            # straight from PSUM P[:, :90] (the dA_r part only shifts it by
            # ~2^-12, and the max-subtract needs only an approximate max
            # for stability), in parallel with the Act-staged fold; the
            # C block is read from PSUM directly (no staging copy).
            # Per-quarter chain, split into stages so four chains can be
            # software-pipelined across the j-groups (DVE executes its
            # stream in-order; un-interleaved chains serialize on their
            # cross-engine latency gaps).  Topic max comes straight from
            # PSUM P[:, :90] (the dA_r part only shifts it by ~2^-12 and
            # stabilization needs only an approximate max), in parallel
            # with the Act-staged fold; the C block is read from PSUM.
            qt = [dict() for _ in range(NTPC)]

            def q_s1(c, j):
                g = gp[c % 2][j]
                Sg = sc_t[c][:, j, 0:SC].rearrange("p (d m) -> p d m", m=M)
                mx = stp.tile([128, D], fp32, tag="qmx", name="qmx")
                nc.vector.tensor_reduce(
                    mx, g[:, 0:SC].rearrange("p (d m) -> p d m", m=M),
                    axis=mybir.AxisListType.X, op=Alu.max)
                tmp = stp.tile([128, SC], fp32, tag="sdl", name="sdl")
                nc.scalar.activation(tmp, g[:, SC:2 * SC], Act.Copy)
                nc.vector.tensor_tensor(sc_t[c][:, j, 0:SC], g[:, 0:SC],
                                        tmp, Alu.add)
                nc.vector.tensor_tensor(
                    Sg, Sg, mx[:, :, None].to_broadcast((128, D, M)),
                    Alu.subtract)
                qt[j]["Sg"] = Sg

            def q_s2(c, j):
                t = c * NTPC + j
                ex = stp.tile([128, D, M], fp32, tag="qex", name="qex")
                nc.scalar.activation(ex, qt[j]["Sg"], Act.Exp,
                                     scale=r_sb[:, t:t + 1])
                qt[j]["ex"] = ex

            def q_s3(c, j):
                g = gp[c % 2][j]
                ex = qt[j]["ex"]
                den = stp.tile([128, D], fp32, tag="qden", name="qden")
                nc.vector.tensor_reduce(den, ex, axis=mybir.AxisListType.X,
                                        op=Alu.add)
                ec = stp.tile([128, D, M], fp32, tag="qec", name="qec")
                nc.vector.tensor_tensor(
                    ec, ex, g[:, 2 * SC:3 * SC].rearrange(
                        "p (d m) -> p d m", m=M), Alu.mult)
                num = stp.tile([128, D], fp32, tag="qnum", name="qnum")
                nc.vector.tensor_reduce(num, ec, axis=mybir.AxisListType.X,
                                        op=Alu.add)
                rden = stp.tile([128, D], fp32, tag="qrden", name="qrden")
                nc.vector.reciprocal(rden, den)
                L = stp.tile([128, D], fp32, tag="qL", name="qL")
                nc.vector.tensor_tensor(L, num, rden, Alu.mult)
                qt[j]["L"] = L

            def q_s4(c, j):
                t = c * NTPC + j
                e2 = stp.tile([128, D], fp32, tag="qe2", name="qe2")
                nc.scalar.activation(e2, qt[j]["L"], Act.Exp,
                                     scale=r_sb[:, t:t + 1])
                qt[j]["e2"] = e2

            def q_s5(c, j):
                t = c * NTPC + j
                e2 = qt[j]["e2"]
                den2 = stp.tile([128, 1], fp32, tag="qden2", name="qden2")
                nc.vector.tensor_reduce(den2, e2, axis=mybir.AxisListType.X,
                                        op=Alu.add)
                rden2 = stp.tile([128, 1], fp32, tag="qrden2",
                                 name="qrden2")
                nc.vector.reciprocal(rden2, den2)
                nc.vector.tensor_scalar_mul(ot_all[:, t, :], e2, rden2)

            c = NCHUNK - 1
            par = c % 2
            for j in range(NTPC):
                nc.tensor.matmul(gp[par][j][:1, :1],
                                 ktr_sb[:, 0, :1].bitcast(fp32),
                                 ktr_sb[:, 0, :1].bitcast(fp32))
            for j in range(NTPC):
                for i in range(KI):
                    nc.tensor.matmul(
                        gp[par][j],
                        ftT[c][:, i, j * 128:(j + 1) * 128],
                        ktr_sb[:, i, :],
                        start=(i == 0), stop=(i == KI - 1))
                if j == 0:
                    softmax_chunk(c - 1)
                elif j == 1:
                    q_s1(c, 0)
                    q_s2(c, 0)
                elif j == 2:
                    q_s1(c, 1)
                    q_s3(c, 0)
                    q_s2(c, 1)
                    q_s4(c, 0)
                else:
                    q_s1(c, 2)
                    q_s3(c, 1)
                    q_s5(c, 0)
                    q_s2(c, 2)
                    q_s4(c, 1)
                    nc.sync.dma_start(outv[:, :NT - 3, :],
                                      ot_all[:, :NT - 3, :])
            q_s1(c, 3)
            q_s3(c, 2)
            q_s5(c, 1)
            q_s2(c, 3)
            q_s4(c, 2)
            q_s3(c, 3)
            q_s5(c, 2)
            q_s4(c, 3)
            q_s5(c, 3)
            nc.sync.dma_start(outv[:, NT - 3:, :], ot_all[:, NT - 3:, :])

    # Post-pass: walrus's codegen rejects instructions with more than one
    # embedded sync wait (S3_LW single-slot limit).  For ANY instruction
    # carrying N>1 waits, hoist N-1 of them into single-wait InstDrain
    # sequencer ops on the same engine immediately before it; the sequencer
    # consumes them in order, so semantics are identical.
    for fn in nc.m.functions:
        for blk in fn.blocks:
            lst = blk.instructions
            k = 0
            while k < len(lst):
                ins = lst[k]
                si = ins.sync_info
                if si is not None and si.on_wait and len(si.on_wait) > 1:
                    w = list(si.on_wait)
                    ups = list(si.on_update or [])
                    ins.sync_info = mybir.SyncInfo(on_wait=[w[-1]],
                                                   on_update=ups)
                    for j, wx in enumerate(w[:-1]):
                        lst.insert(k + j, mybir.InstDrain(
                            name=f"{ins.name}-sw{j}", engine=ins.engine,
                            sync_info=mybir.SyncInfo(on_wait=[wx],
                                                     on_update=[])))
                    k += len(w) - 1
                k += 1

    return nc


def _get_nc():
    if "nc" not in _NC_CACHE:
        _NC_CACHE["nc"] = _build_nc()
    return _NC_CACHE["nc"]


def _rne11(x):
    """Round fp32 to the f32r grid (11 explicit mantissa bits)."""
    xv = np.ascontiguousarray(x, dtype=np.float32).view(np.uint32)
    xv = xv.astype(np.uint64)
    out = ((xv + np.uint64(0x800)) & np.uint64(0xFFFFF000)).astype(np.uint32)
    return out.view(np.float32)


def _host_prep(feature, W_topic, W_domain, memory_tables, category):
    feature = np.ascontiguousarray(np.asarray(feature, dtype=np.float32))
    cat = np.asarray(category).astype(np.int64)
    mems = np.asarray(memory_tables, dtype=np.float32)[cat[:D]]       # [9,10,768]
    mf = mems.reshape(D * M, E).astype(np.float64)
    A = (mf @ np.asarray(W_topic, dtype=np.float64)).astype(np.float32)
    C = (mf @ np.asarray(W_domain, dtype=np.float64)).astype(np.float32)
    Ar = _rne11(A)
    dAr = _rne11(A - Ar)
    Cr = _rne11(C)
    # ktr[p, i, 0:90]=Ar.T, 90:180=dAr.T, 180:270=Cr.T (per i-block of I)
    KT = np.empty((128, KI, KC), dtype=np.float32)
    KT[:, :, 0:SC] = Ar.T.reshape(KI, 128, SC).transpose(1, 0, 2)
    KT[:, :, SC:2 * SC] = dAr.T.reshape(KI, 128, SC).transpose(1, 0, 2)
    KT[:, :, 2 * SC:3 * SC] = Cr.T.reshape(KI, 128, SC).transpose(1, 0, 2)
    KT = np.ascontiguousarray(KT).reshape(128, KI * KC)

    norm = np.sqrt(np.einsum("bi,bi->b", feature, feature,
                             dtype=np.float64))
    r = (TAU / np.maximum(norm, 1e-12)).astype(np.float32)            # [B]
    cst = np.ascontiguousarray(
        r.reshape(NCORES, BLOC // 128, 128).transpose(0, 2, 1))

    # featT[core] = f32r(feature[core block]).T  -> [I, BLOC]
    fr = _rne11(feature)
    featT = np.ascontiguousarray(
        fr.reshape(NCORES, BLOC, I).transpose(0, 2, 1))
    return featT, (cst, KT)


def _run(featT, cstpack, trace=False):
    from concourse.bass_utils import run_bass_kernel_spmd

    cst, KT = cstpack
    nc = _get_nc()
    in_maps = [
        {"feat": featT[c], "cst": cst[c], "ktr": KT}
        for c in range(NCORES)
    ]
    res = run_bass_kernel_spmd(nc, in_maps, core_ids=list(range(NCORES)),
                               trace=trace)
    out = np.concatenate([r["out"] for r in res.results], axis=0)     # [B, 9]
    return out.reshape(B, 1, D), res


def kernel(feature=None, W_topic=None, W_domain=None, memory_tables=None,
           category=None, **_unused):
    featT, cstpack = _host_prep(feature, W_topic, W_domain, memory_tables,
                                category)
    out, _ = _run(featT, cstpack, trace=False)
    return out


# revision 26
# speedup vs baseline: 1.0464x; 1.0283x over previous
"""Trainium2 Bass kernel for nn_MemoryNetwork (scatter_memory).

Reference computation (B=16384, I=2048, E=768, D=9, M=10, TAU=32):
    feat   = feature / ||feature||_2                       [B, I]
    mems_d = memory_tables[category[:9]]                   [D, M, E]  (first-9 quirk)
    t      = feat @ W_topic.T                              [B, E]
    att    = softmax(einsum('be,dme->bdm', t, mems_d)*TAU) [B, D, M]
    sep    = einsum('bdm,dme->bde', att, mems_d)           [B, D, E]
    dproj  = feat @ W_domain.T                             [B, E]
    out    = softmax(einsum('bde,be->bd', sep, dproj)*TAU) [B, 1, D]

Algebraic collapse (exact up to fp reassociation):
    A = mems_d.reshape(90, E) @ W_topic                    [90, I]
    C = mems_d.reshape(90, E) @ W_domain                   [90, I]
    G = feature @ [A; C].T                                 [B, 180]
    r_b = TAU / ||feature[b]||   (folded into the exps as per-row scale)
    s = G[:, :90] (topic logits, groups of 10), c = G[:, 90:]
    topic softmax over m of r*s;  L[b,d] = sum_m att*c;  out = softmax_d(r*L)

Device math runs the big GEMM in float32r (TF32-like, 11-bit mantissa,
1.0 PE cycles/row when the moving free dim is >= 256 -- 4x faster than
fp32).  Raw f32r rounding of both operands costs ~2.3e-2 rel err (over
the 2e-2 gate), so the topic block gets a first-order correction: the
moving tensor is [A_r | dA_r | C_r] (270 cols) where dA_r = f32r(A - A_r),
and s = P[:,:90] + P[:,90:180] is folded during PSUM eviction.  Measured
end-to-end rel err ~1.4e-2.

Per core: 256 matmuls x 270 moving cols = 69,120 PE cycles (~28.8us).
Feature tiles are pre-rounded to the f32r grid AND pre-transposed on the
host (so no PE transposes at all), then streamed as [128, 512] slices
across three DMA queues (sync/HWDGE, scalar/HWDGE, gpsimd/SWDGE).
Sharding: data-parallel over B across 8 cores; K / memory tables are
host-replicated (tiny).
"""

import os
import sys

import numpy as np

for _p in ("/opt/trn_rl_repo", "/root/.axon_site/_ro/trn_rl_repo"):
    if os.path.isdir(_p) and _p not in sys.path:
        sys.path.insert(0, _p)

B, I, E = 16384, 2048, 768
D, M, TAU = 9, 10, 32.0
NCORES = 8
BLOC = B // NCORES          # 2048 rows per core
KI = I // 128               # 16 contraction blocks
SC = D * M                  # 90 = one logit block
KC = 3 * SC                 # 270 moving cols: [A_r | dA_r | C_r]
CHUNK = 512                 # batch-chunk
NCHUNK = BLOC // CHUNK      # 4
NTPC = CHUNK // 128         # 4 b-tiles per chunk
NT = BLOC // 128            # 16 b-tiles per core
CSMALL = NT                 # [r] fp32 const

_NC_CACHE = {}


def _build_nc():
    import concourse.bass as bass
    import concourse.mybir as mybir
    import concourse.tile as tile

    fp32 = mybir.dt.float32
    f32r = mybir.dt.float32r
    Alu = mybir.AluOpType
    Act = mybir.ActivationFunctionType

    nc = bass.Bass()
    # Pre-rounded feature, transposed per core: feat[i*128+p, b] =
    # f32r(feature)[core*BLOC + b, i*128 + p].
    feat = nc.dram_tensor("feat", [KI * 128, BLOC], f32r,
                          kind="ExternalInput")
    # K moving tensor, f32r grid: [128, KI, 270].
    ktr = nc.dram_tensor("ktr", [128, KI * KC], f32r, kind="ExternalInput")
    out = nc.dram_tensor("out", [BLOC, D], fp32, kind="ExternalOutput")

    with tile.TileContext(nc) as tc:
        with (
            tc.tile_pool(name="const", bufs=1) as cpool,
            tc.tile_pool(name="stp", bufs=2) as stp,
            tc.tile_pool(name="pG", bufs=1, space="PSUM") as pG,
        ):
            ot_all = cpool.tile([128, NT, D], fp32)
            ktr_sb = cpool.tile([128, KI, KC], f32r)
            ftT = [cpool.tile([128, KI, CHUNK], f32r, name=f"ftT{c}")
                   for c in range(NCHUNK)]
            sc_t = [cpool.tile([128, NTPC, 2 * SC], fp32, name=f"sc{c}")
                    for c in range(NCHUNK)]
            kv = ktr[:, :].rearrange("p (k c) -> p k c", c=KC)
            wjk = cpool.tile([128, 128], fp32)

            # ---- DMA issue plan ----------------------------------------
            # Cold-start need order: ktr[i] and slice(c0, i) in ascending i
            # (PSUM accumulation makes any i order legal, arrival ~matches
            # ascending).  Queues: sync (HWDGE), scalar (HWDGE), gpsimd
            # (SWDGE, ~1us/DMA descriptor-gen -> few coarse pieces).
            def fslice(c, i0, i1, eng):
                src = feat[i0 * 128:i1 * 128, c * CHUNK:(c + 1) * CHUNK]
                src = src.rearrange("(k p) b -> p k b", p=128)
                eng.dma_start(ftT[c][:, i0:i1, :], src)

            # memset must be the FIRST pool-sequencer instruction or it
            # queues behind the SWDGE descriptor-gens and the PE warm-up
            # never happens.
            nc.gpsimd.memset(wjk[:, :], 0.0)
            KQ = ((0, 1), (1, 2), (2, 4), (4, 6), (6, 8), (8, 10),
                  (10, 12), (12, 14), (14, 16))
            for q0, q1 in KQ:
                nc.sync.dma_start(ktr_sb[:, q0:q1, :], kv[:, q0:q1, :])
            # scalar (= Act engine) only carries pieces needed EARLY: its
            # DMA-queue backpressure convoys the Act compute ops behind it.
            for i0, i1 in ((0, 1), (1, 2), (2, 3), (3, 4), (4, 6)):
                fslice(0, i0, i1, nc.scalar)
            for i0, i1 in ((6, 8), (8, 10), (10, 12), (12, 14)):
                fslice(0, i0, i1, nc.gpsimd)
            fslice(0, 14, 16, nc.sync)
            fslice(1, 0, 4, nc.scalar)
            fslice(1, 4, 8, nc.sync)
            fslice(1, 8, 12, nc.gpsimd)
            fslice(1, 12, 16, nc.gpsimd)
            for c in range(2, NCHUNK):
                fslice(c, 0, 4, nc.sync)
                fslice(c, 4, 8, nc.sync)
                fslice(c, 8, 12, nc.gpsimd)
                fslice(c, 12, 16, nc.scalar)

            # ---- PSUM: 8 banks = 2 chunk-parities x 4 b-tiles ----------
            gp = [[pG.tile([128, KC], fp32, tag=f"gp{par}{j}",
                           name=f"gp{par}{j}") for j in range(NTPC)]
                  for par in range(2)]

            # ---- PE warm-up --------------------------------------------
            # Warm the PE p-state ramp on the memset junk tile (no DMA in
            # the way).  Targets chunk 0's own PSUM bank: the WAW
            # dependency anchors the warm-up BEFORE the first real matmul
            # -- junk with no consumers gets parked late by the scheduler
            # and convoys its engine.  (Multi-wait instructions are handled
            # by the InstDrain post-pass, so no sem-absorber junk needed.)
            for w in range(10):
                nc.tensor.transpose(gp[0][0][:, :128], wjk, wjk)

            def evict(c, j):
                # DVE can read only ONE non-scalar PSUM input per op: stage
                # the dA_r half through SBUF on the (idle) Act engine, which
                # also evicts the C block -- the DVE does only the fold-add
                # (it is the throughput-critical engine).
                g = gp[c % 2][j]
                tmp = stp.tile([128, SC], fp32, tag="sdl", name="sdl")
                nc.scalar.activation(sc_t[c][:, j, SC:2 * SC],
                                     g[:, 2 * SC:3 * SC], Act.Copy)
                nc.scalar.activation(tmp, g[:, SC:2 * SC], Act.Copy)
                nc.vector.tensor_tensor(sc_t[c][:, j, 0:SC], g[:, 0:SC],
                                        tmp, Alu.add)

            def softmax_chunk(ci, j0=0, j1=NTPC):
                sc = sc_t[ci]
                nj = j1 - j0
                S = sc[:, j0:j1, 0:SC].rearrange("p c (d m) -> p c d m", m=M)
                Cv = sc[:, j0:j1, SC:2 * SC].rearrange(
                    "p c (d m) -> p c d m", m=M)
                sh4 = (128, nj, D, M)
                mx = stp.tile([128, nj, D], fp32, tag="mx", name="mx")
                nc.vector.tensor_reduce(mx, S, axis=mybir.AxisListType.X,
                                        op=Alu.max)
                nc.vector.tensor_tensor(
                    S, S, mx[:, :, :, None].to_broadcast(sh4), Alu.subtract)
                ex = stp.tile([128, nj, D, M], fp32, tag="ex", name="ex")
                nc.scalar.activation(ex, S, Act.Exp)
                den = stp.tile([128, nj, D], fp32, tag="den", name="den")
                nc.vector.tensor_reduce(den, ex, axis=mybir.AxisListType.X,
                                        op=Alu.add)
                ec = stp.tile([128, nj, D, M], fp32, tag="ec", name="ec")
                # the elementwise multiply runs on the (otherwise idle)
                # gpsimd engine to keep the DVE stream short
                nc.gpsimd.tensor_tensor(ec, ex, Cv, Alu.mult)
                num = stp.tile([128, nj, D], fp32, tag="num", name="num")
                nc.vector.tensor_reduce(num, ec, axis=mybir.AxisListType.X,
                                        op=Alu.add)
                rden = stp.tile([128, nj, D], fp32, tag="rden", name="rden")
                nc.vector.reciprocal(rden, den)
                L = stp.tile([128, nj, D], fp32, tag="L", name="L")
                nc.vector.tensor_tensor(L, num, rden, Alu.mult)
                sh3 = (128, nj, D)
                # domain softmax: r*L stays within +-~85, exp() cannot
                # overflow fp32 -- skip the max-subtraction
                e2 = stp.tile([128, nj, D], fp32, tag="e2", name="e2")
                nc.scalar.activation(e2, L, Act.Exp)
                den2 = stp.tile([128, nj], fp32, tag="den2", name="den2")
                nc.vector.tensor_reduce(den2, e2, axis=mybir.AxisListType.X,
                                        op=Alu.add)
                rden2 = stp.tile([128, nj], fp32, tag="rden2", name="rden2")
                nc.vector.reciprocal(rden2, den2)
                nc.vector.tensor_tensor(
                    ot_all[:, ci * NTPC + j0:ci * NTPC + j1, :], e2,
                    rden2[:, :, None].to_broadcast(sh3), Alu.mult)

            outv = out[:, :].rearrange("(t p) d -> p t d", p=128)

            # ---- chunks 0-2: i-outer (stream slices as they arrive) ----
            for c in range(NCHUNK - 1):
                par = c % 2
                if c >= 2:
                    # absorb the PSUM WAR-release wait (vs chunk c-2's
                    # eviction) into junk matmuls, one per bank
                    for j in range(NTPC):
                        nc.tensor.matmul(gp[par][j][:1, :1],
                                         ktr_sb[:, 0, :1].bitcast(fp32),
                                         ktr_sb[:, 0, :1].bitcast(fp32))
                for i in range(KI):
                    for j in range(NTPC):
                        nc.tensor.matmul(
                            gp[par][j],
                            ftT[c][:, i, j * 128:(j + 1) * 128],
                            ktr_sb[:, i, :],
                            start=(i == 0), stop=(i == KI - 1))
                if c > 0:
                    # previous chunk's softmax overlaps this chunk's PE work
                    softmax_chunk(c - 1)
                for j in range(NTPC):
                    evict(c, j)

            # ---- last chunk: j-outer so evictions/softmax pipeline -----
            # Tail-latency-optimized per-b-tile chain: topic max comes
            # straight from PSUM P[:, :90] (the dA_r part only shifts it by
            # ~2^-12, and the max-subtract needs only an approximate max
            # for stability), in parallel with the Act-staged fold; the
            # C block is read from PSUM directly (no staging copy).
            # Per-quarter chain, split into stages so four chains can be
            # software-pipelined across the j-groups (DVE executes its
            # stream in-order; un-interleaved chains serialize on their
            # cross-engine latency gaps).  Topic max comes straight from
            # PSUM P[:, :90] (the dA_r part only shifts it by ~2^-12 and
            # stabilization needs only an approximate max), in parallel
            # with the Act-staged fold; the C block is read from PSUM.
            qt = [dict() for _ in range(NTPC)]

            def q_s1(c, j):
                g = gp[c % 2][j]
                Sg = sc_t[c][:, j, 0:SC].rearrange("p (d m) -> p d m", m=M)
                mx = stp.tile([128, D], fp32, tag="qmx", name="qmx")
                nc.vector.tensor_reduce(
                    mx, g[:, 0:SC].rearrange("p (d m) -> p d m", m=M),
                    axis=mybir.AxisListType.X, op=Alu.max)
                tmp = stp.tile([128, SC], fp32, tag="sdl", name="sdl")
                nc.scalar.activation(tmp, g[:, SC:2 * SC], Act.Copy)
                nc.vector.tensor_tensor(sc_t[c][:, j, 0:SC], g[:, 0:SC],
                                        tmp, Alu.add)
                nc.vector.tensor_tensor(
                    Sg, Sg, mx[:, :, None].to_broadcast((128, D, M)),
                    Alu.subtract)
                qt[j]["Sg"] = Sg

            def q_s2(c, j):
                ex = stp.tile([128, D, M], fp32, tag="qex", name="qex")
                nc.scalar.activation(ex, qt[j]["Sg"], Act.Exp)
                qt[j]["ex"] = ex

            def q_s3(c, j):
                g = gp[c % 2][j]
                ex = qt[j]["ex"]
                den = stp.tile([128, D], fp32, tag="qden", name="qden")
                nc.vector.tensor_reduce(den, ex, axis=mybir.AxisListType.X,
                                        op=Alu.add)
                ec = stp.tile([128, D, M], fp32, tag="qec", name="qec")
                nc.vector.tensor_tensor(
                    ec, ex, g[:, 2 * SC:3 * SC].rearrange(
                        "p (d m) -> p d m", m=M), Alu.mult)
                num = stp.tile([128, D], fp32, tag="qnum", name="qnum")
                nc.vector.tensor_reduce(num, ec, axis=mybir.AxisListType.X,
                                        op=Alu.add)
                rden = stp.tile([128, D], fp32, tag="qrden", name="qrden")
                nc.vector.reciprocal(rden, den)
                L = stp.tile([128, D], fp32, tag="qL", name="qL")
                nc.vector.tensor_tensor(L, num, rden, Alu.mult)
                qt[j]["L"] = L

            def q_s4(c, j):
                e2 = stp.tile([128, D], fp32, tag="qe2", name="qe2")
                nc.scalar.activation(e2, qt[j]["L"], Act.Exp)
                qt[j]["e2"] = e2

            def q_s5(c, j):
                t = c * NTPC + j
                e2 = qt[j]["e2"]
                den2 = stp.tile([128, 1], fp32, tag="qden2", name="qden2")
                nc.vector.tensor_reduce(den2, e2, axis=mybir.AxisListType.X,
                                        op=Alu.add)
                rden2 = stp.tile([128, 1], fp32, tag="qrden2",
                                 name="qrden2")
                nc.vector.reciprocal(rden2, den2)
                nc.vector.tensor_scalar_mul(ot_all[:, t, :], e2, rden2)

            c = NCHUNK - 1
            par = c % 2
            for j in range(NTPC):
                nc.tensor.matmul(gp[par][j][:1, :1],
                                 ktr_sb[:, 0, :1].bitcast(fp32),
                                 ktr_sb[:, 0, :1].bitcast(fp32))
            for j in range(NTPC):
                for i in range(KI):
                    nc.tensor.matmul(
                        gp[par][j],
                        ftT[c][:, i, j * 128:(j + 1) * 128],
                        ktr_sb[:, i, :],
                        start=(i == 0), stop=(i == KI - 1))
                if j == 0:
                    softmax_chunk(c - 1)
                elif j == 1:
                    q_s1(c, 0)
                    q_s2(c, 0)
                elif j == 2:
                    q_s1(c, 1)
                    q_s3(c, 0)
                    q_s2(c, 1)
                    q_s4(c, 0)
                else:
                    q_s1(c, 2)
                    q_s3(c, 1)
                    q_s5(c, 0)
                    q_s2(c, 2)
                    q_s4(c, 1)
                    nc.sync.dma_start(outv[:, :NT - 3, :],
                                      ot_all[:, :NT - 3, :])
            q_s1(c, 3)
            q_s3(c, 2)
            q_s5(c, 1)
            q_s2(c, 3)
            q_s4(c, 2)
            q_s3(c, 3)
            q_s5(c, 2)
            q_s4(c, 3)
            q_s5(c, 3)
            nc.sync.dma_start(outv[:, NT - 3:, :], ot_all[:, NT - 3:, :])

    # Post-pass: walrus's codegen rejects instructions with more than one
    # embedded sync wait (S3_LW single-slot limit).  For ANY instruction
    # carrying N>1 waits, hoist N-1 of them into single-wait InstDrain
    # sequencer ops on the same engine immediately before it; the sequencer
    # consumes them in order, so semantics are identical.
    for fn in nc.m.functions:
        for blk in fn.blocks:
            lst = blk.instructions
            k = 0
            while k < len(lst):
                ins = lst[k]
                si = ins.sync_info
                if si is not None and si.on_wait and len(si.on_wait) > 1:
                    w = list(si.on_wait)
                    ups = list(si.on_update or [])
                    ins.sync_info = mybir.SyncInfo(on_wait=[w[-1]],
                                                   on_update=ups)
                    for j, wx in enumerate(w[:-1]):
                        lst.insert(k + j, mybir.InstDrain(
                            name=f"{ins.name}-sw{j}", engine=ins.engine,
                            sync_info=mybir.SyncInfo(on_wait=[wx],
                                                     on_update=[])))
                    k += len(w) - 1
                k += 1

    return nc


def _get_nc():
    if "nc" not in _NC_CACHE:
        _NC_CACHE["nc"] = _build_nc()
    return _NC_CACHE["nc"]


def _rne11(x):
    """Round fp32 to the f32r grid (11 explicit mantissa bits)."""
    xv = np.ascontiguousarray(x, dtype=np.float32).view(np.uint32)
    xv = xv.astype(np.uint64)
    out = ((xv + np.uint64(0x800)) & np.uint64(0xFFFFF000)).astype(np.uint32)
    return out.view(np.float32)


def _host_prep(feature, W_topic, W_domain, memory_tables, category):
    feature = np.ascontiguousarray(np.asarray(feature, dtype=np.float32))
    cat = np.asarray(category).astype(np.int64)
    mems = np.asarray(memory_tables, dtype=np.float32)[cat[:D]]       # [9,10,768]
    mf = mems.reshape(D * M, E).astype(np.float64)
    A = (mf @ np.asarray(W_topic, dtype=np.float64)).astype(np.float32)
    C = (mf @ np.asarray(W_domain, dtype=np.float64)).astype(np.float32)
    Ar = _rne11(A)
    dAr = _rne11(A - Ar)
    Cr = _rne11(C)
    # ktr[p, i, 0:90]=Ar.T, 90:180=dAr.T, 180:270=Cr.T (per i-block of I)
    KT = np.empty((128, KI, KC), dtype=np.float32)
    KT[:, :, 0:SC] = Ar.T.reshape(KI, 128, SC).transpose(1, 0, 2)
    KT[:, :, SC:2 * SC] = dAr.T.reshape(KI, 128, SC).transpose(1, 0, 2)
    KT[:, :, 2 * SC:3 * SC] = Cr.T.reshape(KI, 128, SC).transpose(1, 0, 2)
    KT = np.ascontiguousarray(KT).reshape(128, KI * KC)

    norm = np.sqrt(np.einsum("bi,bi->b", feature, feature,
                             dtype=np.float64))
    r = (TAU / np.maximum(norm, 1e-12)).astype(np.float32)            # [B]
    # fold the per-row scale r into the feature itself: both softmax
    # stages are scale-equivariant in r (s, c, and L all carry exactly
    # one factor of r), so no on-device scaling is needed at all
    fr = _rne11(feature * r[:, None])
    featT = np.ascontiguousarray(
        fr.reshape(NCORES, BLOC, I).transpose(0, 2, 1))
    return featT, KT


def _run(featT, KT, trace=False):
    from concourse.bass_utils import run_bass_kernel_spmd

    nc = _get_nc()
    in_maps = [
        {"feat": featT[c], "ktr": KT}
        for c in range(NCORES)
    ]
    res = run_bass_kernel_spmd(nc, in_maps, core_ids=list(range(NCORES)),
                               trace=trace)
    out = np.concatenate([r["out"] for r in res.results], axis=0)     # [B, 9]
    return out.reshape(B, 1, D), res


def kernel(feature=None, W_topic=None, W_domain=None, memory_tables=None,
           category=None, **_unused):
    featT, KT = _host_prep(feature, W_topic, W_domain, memory_tables,
                           category)
    out, _ = _run(featT, KT, trace=False)
    return out


# revision 27
# speedup vs baseline: 1.0496x; 1.0031x over previous
"""Trainium2 Bass kernel for nn_MemoryNetwork (scatter_memory).

Reference computation (B=16384, I=2048, E=768, D=9, M=10, TAU=32):
    feat   = feature / ||feature||_2                       [B, I]
    mems_d = memory_tables[category[:9]]                   [D, M, E]  (first-9 quirk)
    t      = feat @ W_topic.T                              [B, E]
    att    = softmax(einsum('be,dme->bdm', t, mems_d)*TAU) [B, D, M]
    sep    = einsum('bdm,dme->bde', att, mems_d)           [B, D, E]
    dproj  = feat @ W_domain.T                             [B, E]
    out    = softmax(einsum('bde,be->bd', sep, dproj)*TAU) [B, 1, D]

Algebraic collapse (exact up to fp reassociation):
    A = mems_d.reshape(90, E) @ W_topic                    [90, I]
    C = mems_d.reshape(90, E) @ W_domain                   [90, I]
    G = feature @ [A; C].T                                 [B, 180]
    r_b = TAU / ||feature[b]||   (folded into the exps as per-row scale)
    s = G[:, :90] (topic logits, groups of 10), c = G[:, 90:]
    topic softmax over m of r*s;  L[b,d] = sum_m att*c;  out = softmax_d(r*L)

Device math runs the big GEMM in float32r (TF32-like, 11-bit mantissa,
1.0 PE cycles/row when the moving free dim is >= 256 -- 4x faster than
fp32).  Raw f32r rounding of both operands costs ~2.3e-2 rel err (over
the 2e-2 gate), so the topic block gets a first-order correction: the
moving tensor is [A_r | dA_r | C_r] (270 cols) where dA_r = f32r(A - A_r),
and s = P[:,:90] + P[:,90:180] is folded during PSUM eviction.  Measured
end-to-end rel err ~1.4e-2.

Per core: 256 matmuls x 270 moving cols = 69,120 PE cycles (~28.8us).
Feature tiles are pre-rounded to the f32r grid AND pre-transposed on the
host (so no PE transposes at all), then streamed as [128, 512] slices
across three DMA queues (sync/HWDGE, scalar/HWDGE, gpsimd/SWDGE).
Sharding: data-parallel over B across 8 cores; K / memory tables are
host-replicated (tiny).
"""

import os
import sys

import numpy as np

for _p in ("/opt/trn_rl_repo", "/root/.axon_site/_ro/trn_rl_repo"):
    if os.path.isdir(_p) and _p not in sys.path:
        sys.path.insert(0, _p)

B, I, E = 16384, 2048, 768
D, M, TAU = 9, 10, 32.0
NCORES = 8
BLOC = B // NCORES          # 2048 rows per core
KI = I // 128               # 16 contraction blocks
SC = D * M                  # 90 = one logit block
KC = 3 * SC                 # 270 moving cols: [A_r | dA_r | C_r]
CHUNK = 512                 # batch-chunk
NCHUNK = BLOC // CHUNK      # 4
NTPC = CHUNK // 128         # 4 b-tiles per chunk
NT = BLOC // 128            # 16 b-tiles per core
CSMALL = NT                 # [r] fp32 const

_NC_CACHE = {}


def _build_nc():
    import concourse.bass as bass
    import concourse.mybir as mybir
    import concourse.tile as tile

    fp32 = mybir.dt.float32
    f32r = mybir.dt.float32r
    Alu = mybir.AluOpType
    Act = mybir.ActivationFunctionType

    nc = bass.Bass()
    # Pre-rounded feature, transposed per core: feat[i*128+p, b] =
    # f32r(feature)[core*BLOC + b, i*128 + p].
    feat = nc.dram_tensor("feat", [KI * 128, BLOC], f32r,
                          kind="ExternalInput")
    # K moving tensor, f32r grid: [128, KI, 270].
    ktr = nc.dram_tensor("ktr", [128, KI * KC], f32r, kind="ExternalInput")
    # Small fp32 consts: r[p, t] = TAU/||feature[t*128+p]||.
    cst = nc.dram_tensor("cst", [128, CSMALL], fp32, kind="ExternalInput")
    out = nc.dram_tensor("out", [BLOC, D], fp32, kind="ExternalOutput")

    with tile.TileContext(nc) as tc:
        with (
            tc.tile_pool(name="const", bufs=1) as cpool,
            tc.tile_pool(name="stp", bufs=2) as stp,
            tc.tile_pool(name="pG", bufs=1, space="PSUM") as pG,
        ):
            ot_all = cpool.tile([128, NT, D], fp32)
            cst_sb = cpool.tile([128, CSMALL], fp32)
            ktr_sb = cpool.tile([128, KI, KC], f32r)
            ftT = [cpool.tile([128, KI, CHUNK], f32r, name=f"ftT{c}")
                   for c in range(NCHUNK)]
            sc_t = [cpool.tile([128, NTPC, 2 * SC], fp32, name=f"sc{c}")
                    for c in range(NCHUNK)]
            r_sb = cst_sb[:, :NT]
            kv = ktr[:, :].rearrange("p (k c) -> p k c", c=KC)
            wjk = cpool.tile([128, 128], fp32)

            # ---- DMA issue plan ----------------------------------------
            # Cold-start need order: ktr[i] and slice(c0, i) in ascending i
            # (PSUM accumulation makes any i order legal, arrival ~matches
            # ascending).  Queues: sync (HWDGE), scalar (HWDGE), gpsimd
            # (SWDGE, ~1us/DMA descriptor-gen -> few coarse pieces).
            def fslice(c, i0, i1, eng):
                src = feat[i0 * 128:i1 * 128, c * CHUNK:(c + 1) * CHUNK]
                src = src.rearrange("(k p) b -> p k b", p=128)
                eng.dma_start(ftT[c][:, i0:i1, :], src)

            # memset must be the FIRST pool-sequencer instruction or it
            # queues behind the SWDGE descriptor-gens and the PE warm-up
            # never happens.
            nc.gpsimd.memset(wjk[:, :], 0.0)
            nc.sync.dma_start(cst_sb, cst[:, :])
            KQ = ((0, 1), (1, 2), (2, 4), (4, 6), (6, 8), (8, 10),
                  (10, 12), (12, 14), (14, 16))
            for q0, q1 in KQ:
                nc.sync.dma_start(ktr_sb[:, q0:q1, :], kv[:, q0:q1, :])
            # scalar (= Act engine) only carries pieces needed EARLY: its
            # DMA-queue backpressure convoys the Act compute ops behind it.
            for i0, i1 in ((0, 1), (1, 2), (2, 3), (3, 4), (4, 6)):
                fslice(0, i0, i1, nc.scalar)
            for i0, i1 in ((6, 8), (8, 10), (10, 12), (12, 14)):
                fslice(0, i0, i1, nc.gpsimd)
            fslice(0, 14, 16, nc.sync)
            fslice(1, 0, 4, nc.scalar)
            fslice(1, 4, 8, nc.sync)
            fslice(1, 8, 12, nc.gpsimd)
            fslice(1, 12, 16, nc.gpsimd)
            for c in range(2, NCHUNK):
                fslice(c, 0, 4, nc.sync)
                fslice(c, 4, 8, nc.sync)
                fslice(c, 8, 12, nc.gpsimd)
                fslice(c, 12, 16, nc.scalar)

            # ---- PSUM: 8 banks = 2 chunk-parities x 4 b-tiles ----------
            gp = [[pG.tile([128, KC], fp32, tag=f"gp{par}{j}",
                           name=f"gp{par}{j}") for j in range(NTPC)]
                  for par in range(2)]

            # ---- PE warm-up --------------------------------------------
            # Warm the PE p-state ramp on the memset junk tile (no DMA in
            # the way).  Targets chunk 0's own PSUM bank: the WAW
            # dependency anchors the warm-up BEFORE the first real matmul
            # -- junk with no consumers gets parked late by the scheduler
            # and convoys its engine.  (Multi-wait instructions are handled
            # by the InstDrain post-pass, so no sem-absorber junk needed.)
            for w in range(10):
                nc.tensor.transpose(gp[0][0][:, :128], wjk, wjk)

            def evict(c, j):
                # DVE can read only ONE non-scalar PSUM input per op: stage
                # the dA_r half through SBUF on the (idle) Act engine, which
                # also evicts the C block; both Act copies fold in the
                # per-row scale r, and the DVE does one fused r*P1 + tmp.
                g = gp[c % 2][j]
                rt = r_sb[:, c * NTPC + j:c * NTPC + j + 1]
                tmp = stp.tile([128, SC], fp32, tag="sdl", name="sdl")
                nc.scalar.activation(sc_t[c][:, j, SC:2 * SC],
                                     g[:, 2 * SC:3 * SC], Act.Copy,
                                     scale=rt)
                nc.scalar.activation(tmp, g[:, SC:2 * SC], Act.Copy,
                                     scale=rt)
                nc.vector.scalar_tensor_tensor(
                    sc_t[c][:, j, 0:SC], g[:, 0:SC], rt, tmp,
                    Alu.mult, Alu.add)

            def softmax_chunk(ci, j0=0, j1=NTPC):
                sc = sc_t[ci]
                nj = j1 - j0
                S = sc[:, j0:j1, 0:SC].rearrange("p c (d m) -> p c d m", m=M)
                Cv = sc[:, j0:j1, SC:2 * SC].rearrange(
                    "p c (d m) -> p c d m", m=M)
                sh4 = (128, nj, D, M)
                mx = stp.tile([128, nj, D], fp32, tag="mx", name="mx")
                nc.vector.tensor_reduce(mx, S, axis=mybir.AxisListType.X,
                                        op=Alu.max)
                nc.vector.tensor_tensor(
                    S, S, mx[:, :, :, None].to_broadcast(sh4), Alu.subtract)
                ex = stp.tile([128, nj, D, M], fp32, tag="ex", name="ex")
                nc.scalar.activation(ex, S, Act.Exp)
                den = stp.tile([128, nj, D], fp32, tag="den", name="den")
                nc.vector.tensor_reduce(den, ex, axis=mybir.AxisListType.X,
                                        op=Alu.add)
                ec = stp.tile([128, nj, D, M], fp32, tag="ec", name="ec")
                # the elementwise multiply runs on the (otherwise idle)
                # gpsimd engine to keep the DVE stream short
                nc.gpsimd.tensor_tensor(ec, ex, Cv, Alu.mult)
                num = stp.tile([128, nj, D], fp32, tag="num", name="num")
                nc.vector.tensor_reduce(num, ec, axis=mybir.AxisListType.X,
                                        op=Alu.add)
                rden = stp.tile([128, nj, D], fp32, tag="rden", name="rden")
                nc.vector.reciprocal(rden, den)
                L = stp.tile([128, nj, D], fp32, tag="L", name="L")
                nc.vector.tensor_tensor(L, num, rden, Alu.mult)
                sh3 = (128, nj, D)
                # domain softmax: r*L stays within +-~85, exp() cannot
                # overflow fp32 -- skip the max-subtraction
                e2 = stp.tile([128, nj, D], fp32, tag="e2", name="e2")
                nc.scalar.activation(e2, L, Act.Exp)
                den2 = stp.tile([128, nj], fp32, tag="den2", name="den2")
                nc.vector.tensor_reduce(den2, e2, axis=mybir.AxisListType.X,
                                        op=Alu.add)
                rden2 = stp.tile([128, nj], fp32, tag="rden2", name="rden2")
                nc.vector.reciprocal(rden2, den2)
                nc.vector.tensor_tensor(
                    ot_all[:, ci * NTPC + j0:ci * NTPC + j1, :], e2,
                    rden2[:, :, None].to_broadcast(sh3), Alu.mult)

            outv = out[:, :].rearrange("(t p) d -> p t d", p=128)

            # ---- chunks 0-2: i-outer (stream slices as they arrive) ----
            for c in range(NCHUNK - 1):
                par = c % 2
                if c >= 2:
                    # absorb the PSUM WAR-release wait (vs chunk c-2's
                    # eviction) into junk matmuls, one per bank
                    for j in range(NTPC):
                        nc.tensor.matmul(gp[par][j][:1, :1],
                                         ktr_sb[:, 0, :1].bitcast(fp32),
                                         ktr_sb[:, 0, :1].bitcast(fp32))
                for i in range(KI):
                    for j in range(NTPC):
                        nc.tensor.matmul(
                            gp[par][j],
                            ftT[c][:, i, j * 128:(j + 1) * 128],
                            ktr_sb[:, i, :],
                            start=(i == 0), stop=(i == KI - 1))
                if c > 0:
                    # previous chunk's softmax overlaps this chunk's PE work
                    softmax_chunk(c - 1)
                for j in range(NTPC):
                    evict(c, j)

            # ---- last chunk: j-outer so evictions/softmax pipeline -----
            # Tail-latency-optimized per-b-tile chain: topic max comes
            # straight from PSUM P[:, :90] (the dA_r part only shifts it by
            # ~2^-12, and the max-subtract needs only an approximate max
            # for stability), in parallel with the Act-staged fold; the
            # C block is read from PSUM directly (no staging copy).
            # Per-quarter chain, split into stages so four chains can be
            # software-pipelined across the j-groups (DVE executes its
            # stream in-order; un-interleaved chains serialize on their
            # cross-engine latency gaps).  Topic max comes straight from
            # PSUM P[:, :90] (the dA_r part only shifts it by ~2^-12 and
            # stabilization needs only an approximate max), in parallel
            # with the Act-staged fold; the C block is read from PSUM.
            qt = [dict() for _ in range(NTPC)]

            def q_s1(c, j):
                g = gp[c % 2][j]
                rt = r_sb[:, c * NTPC + j:c * NTPC + j + 1]
                Sg = sc_t[c][:, j, 0:SC].rearrange("p (d m) -> p d m", m=M)
                tmp = stp.tile([128, SC], fp32, tag="sdl", name="sdl")
                nc.scalar.activation(tmp, g[:, SC:2 * SC], Act.Copy,
                                     scale=rt)
                nc.vector.scalar_tensor_tensor(
                    sc_t[c][:, j, 0:SC], g[:, 0:SC], rt, tmp,
                    Alu.mult, Alu.add)
                mx = stp.tile([128, D], fp32, tag="qmx", name="qmx")
                nc.vector.tensor_reduce(mx, Sg, axis=mybir.AxisListType.X,
                                        op=Alu.max)
                nc.vector.tensor_tensor(
                    Sg, Sg, mx[:, :, None].to_broadcast((128, D, M)),
                    Alu.subtract)
                qt[j]["Sg"] = Sg

            def q_s2(c, j):
                ex = stp.tile([128, D, M], fp32, tag="qex", name="qex")
                nc.scalar.activation(ex, qt[j]["Sg"], Act.Exp)
                qt[j]["ex"] = ex

            def q_s3(c, j):
                g = gp[c % 2][j]
                ex = qt[j]["ex"]
                den = stp.tile([128, D], fp32, tag="qden", name="qden")
                nc.vector.tensor_reduce(den, ex, axis=mybir.AxisListType.X,
                                        op=Alu.add)
                rt = r_sb[:, c * NTPC + j:c * NTPC + j + 1]
                ec = stp.tile([128, D, M], fp32, tag="qec", name="qec")
                nc.vector.scalar_tensor_tensor(
                    ec, g[:, 2 * SC:3 * SC].rearrange(
                        "p (d m) -> p d m", m=M), rt, ex,
                    Alu.mult, Alu.mult)
                num = stp.tile([128, D], fp32, tag="qnum", name="qnum")
                nc.vector.tensor_reduce(num, ec, axis=mybir.AxisListType.X,
                                        op=Alu.add)
                rden = stp.tile([128, D], fp32, tag="qrden", name="qrden")
                nc.vector.reciprocal(rden, den)
                L = stp.tile([128, D], fp32, tag="qL", name="qL")
                nc.vector.tensor_tensor(L, num, rden, Alu.mult)
                qt[j]["L"] = L

            def q_s4(c, j):
                e2 = stp.tile([128, D], fp32, tag="qe2", name="qe2")
                nc.scalar.activation(e2, qt[j]["L"], Act.Exp)
                qt[j]["e2"] = e2

            def q_s5(c, j):
                t = c * NTPC + j
                e2 = qt[j]["e2"]
                den2 = stp.tile([128, 1], fp32, tag="qden2", name="qden2")
                nc.vector.tensor_reduce(den2, e2, axis=mybir.AxisListType.X,
                                        op=Alu.add)
                rden2 = stp.tile([128, 1], fp32, tag="qrden2",
                                 name="qrden2")
                nc.vector.reciprocal(rden2, den2)
                nc.vector.tensor_scalar_mul(ot_all[:, t, :], e2, rden2)

            c = NCHUNK - 1
            par = c % 2
            for j in range(NTPC):
                nc.tensor.matmul(gp[par][j][:1, :1],
                                 ktr_sb[:, 0, :1].bitcast(fp32),
                                 ktr_sb[:, 0, :1].bitcast(fp32))
            for j in range(NTPC):
                for i in range(KI):
                    nc.tensor.matmul(
                        gp[par][j],
                        ftT[c][:, i, j * 128:(j + 1) * 128],
                        ktr_sb[:, i, :],
                        start=(i == 0), stop=(i == KI - 1))
                if j == 0:
                    softmax_chunk(c - 1)
                elif j == 1:
                    q_s1(c, 0)
                    q_s2(c, 0)
                elif j == 2:
                    q_s1(c, 1)
                    q_s3(c, 0)
                    q_s2(c, 1)
                    q_s4(c, 0)
                else:
                    q_s1(c, 2)
                    q_s3(c, 1)
                    q_s5(c, 0)
                    q_s2(c, 2)
                    q_s4(c, 1)
                    nc.sync.dma_start(outv[:, :NT - 3, :],
                                      ot_all[:, :NT - 3, :])
            q_s1(c, 3)
            q_s3(c, 2)
            q_s5(c, 1)
            q_s2(c, 3)
            q_s4(c, 2)
            q_s3(c, 3)
            q_s5(c, 2)
            q_s4(c, 3)
            q_s5(c, 3)
            nc.sync.dma_start(outv[:, NT - 3:, :], ot_all[:, NT - 3:, :])

    # Post-pass: walrus's codegen rejects instructions with more than one
    # embedded sync wait (S3_LW single-slot limit).  For ANY instruction
    # carrying N>1 waits, hoist N-1 of them into single-wait InstDrain
    # sequencer ops on the same engine immediately before it; the sequencer
    # consumes them in order, so semantics are identical.
    for fn in nc.m.functions:
        for blk in fn.blocks:
            lst = blk.instructions
            k = 0
            while k < len(lst):
                ins = lst[k]
                si = ins.sync_info
                if si is not None and si.on_wait and len(si.on_wait) > 1:
                    w = list(si.on_wait)
                    ups = list(si.on_update or [])
                    ins.sync_info = mybir.SyncInfo(on_wait=[w[-1]],
                                                   on_update=ups)
                    for j, wx in enumerate(w[:-1]):
                        lst.insert(k + j, mybir.InstDrain(
                            name=f"{ins.name}-sw{j}", engine=ins.engine,
                            sync_info=mybir.SyncInfo(on_wait=[wx],
                                                     on_update=[])))
                    k += len(w) - 1
                k += 1

    return nc


def _get_nc():
    if "nc" not in _NC_CACHE:
        _NC_CACHE["nc"] = _build_nc()
    return _NC_CACHE["nc"]


def _rne11(x):
    """Round fp32 to the f32r grid (11 explicit mantissa bits)."""
    xv = np.ascontiguousarray(x, dtype=np.float32).view(np.uint32)
    xv = xv.astype(np.uint64)
    out = ((xv + np.uint64(0x800)) & np.uint64(0xFFFFF000)).astype(np.uint32)
    return out.view(np.float32)


def _host_prep(feature, W_topic, W_domain, memory_tables, category):
    feature = np.ascontiguousarray(np.asarray(feature, dtype=np.float32))
    cat = np.asarray(category).astype(np.int64)
    mems = np.asarray(memory_tables, dtype=np.float32)[cat[:D]]       # [9,10,768]
    mf = mems.reshape(D * M, E).astype(np.float64)
    A = (mf @ np.asarray(W_topic, dtype=np.float64)).astype(np.float32)
    C = (mf @ np.asarray(W_domain, dtype=np.float64)).astype(np.float32)
    Ar = _rne11(A)
    dAr = _rne11(A - Ar)
    Cr = _rne11(C)
    # ktr[p, i, 0:90]=Ar.T, 90:180=dAr.T, 180:270=Cr.T (per i-block of I)
    KT = np.empty((128, KI, KC), dtype=np.float32)
    KT[:, :, 0:SC] = Ar.T.reshape(KI, 128, SC).transpose(1, 0, 2)
    KT[:, :, SC:2 * SC] = dAr.T.reshape(KI, 128, SC).transpose(1, 0, 2)
    KT[:, :, 2 * SC:3 * SC] = Cr.T.reshape(KI, 128, SC).transpose(1, 0, 2)
    KT = np.ascontiguousarray(KT).reshape(128, KI * KC)

    norm = np.sqrt(np.einsum("bi,bi->b", feature, feature,
                             dtype=np.float64))
    r = (TAU / np.maximum(norm, 1e-12)).astype(np.float32)            # [B]
    cst = np.ascontiguousarray(
        r.reshape(NCORES, BLOC // 128, 128).transpose(0, 2, 1))
    fr = _rne11(feature)
    featT = np.ascontiguousarray(
        fr.reshape(NCORES, BLOC, I).transpose(0, 2, 1))
    return featT, (cst, KT)


def _run(featT, cstpack, trace=False):
    from concourse.bass_utils import run_bass_kernel_spmd

    cst, KT = cstpack
    nc = _get_nc()
    in_maps = [
        {"feat": featT[c], "cst": cst[c], "ktr": KT}
        for c in range(NCORES)
    ]
    res = run_bass_kernel_spmd(nc, in_maps, core_ids=list(range(NCORES)),
                               trace=trace)
    out = np.concatenate([r["out"] for r in res.results], axis=0)     # [B, 9]
    return out.reshape(B, 1, D), res


def kernel(feature=None, W_topic=None, W_domain=None, memory_tables=None,
           category=None, **_unused):
    featT, cstpack = _host_prep(feature, W_topic, W_domain, memory_tables,
                                category)
    out, _ = _run(featT, cstpack, trace=False)
    return out
